# revision 1
# baseline (speedup 1.0000x reference)
"""3D window attention (B=32, N=513, D=768, H=12) on 8 trn2 NeuronCores.

Data-parallel over batch (4 per core). Per-core Bass/Tile kernel:
  A:  x -> xT via PE transposes
  B:  v = x @ Wv^T   (token-major, per-head 65-col groups with a ones column)
  B2: qT/kT = (Wq x^T, Wk x^T) head-pair-major, spilled to HBM scratch
  C:  per head-pair g, batch b: S^T = bias-inject + K Q^T (row-group packed),
      exp on ACT, U = [v;1]^T P on PE (softmax sums ride along), normalize
  D:  y = a @ Wp^T + b_proj
All matmuls run as float32r (TF32-like). Bias is injected via a bf16
identity matmul into PSUM so no elementwise bias pass is needed.
"""

import numpy as np

B, N_TOK, DIM, HEADS = 32, 513, 768, 12
HD = DIM // HEADS           # 64
SCALE = HD ** -0.5
N_CORES = 8
B_PER = B // N_CORES        # 4
T = B_PER * N_TOK           # 2052
G = HEADS // 2              # 6 head pairs
MT = 5                      # m tiles per batch (513 -> 5*128 padded)
SEC_K = 640                 # padded keys per batch
NEG = -1e30

_CACHE = {}


def _build_nc():
    import os
    import concourse.bacc as bacc
    import concourse.mybir as mybir
    import concourse.tile as tile

    F32 = mybir.dt.float32
    F32R = mybir.dt.float32r
    BF16 = mybir.dt.bfloat16
    AF = mybir.ActivationFunctionType

    nc = bacc.Bacc("TRN2", target_bir_lowering=False, debug=False)

    x_d = nc.dram_tensor("x", [B_PER, N_TOK, DIM], F32R, kind="ExternalInput")
    wqk_d = nc.dram_tensor("wqkT", [DIM, 2 * DIM], F32R, kind="ExternalInput")
    wv_d = nc.dram_tensor("wvT", [DIM, DIM], F32R, kind="ExternalInput")
    wp_d = nc.dram_tensor("wpT", [DIM, DIM], F32R, kind="ExternalInput")
    bT_d = nc.dram_tensor("bT", [HEADS, 128, MT, N_TOK], BF16, kind="ExternalInput")
    id32_d = nc.dram_tensor("id32", [128, 128], F32R, kind="ExternalInput")
    id16_d = nc.dram_tensor("id16", [128, 128], BF16, kind="ExternalInput")
    bb_d = nc.dram_tensor("bb", [128, DIM], F32, kind="ExternalInput")
    ebt_d = nc.dram_tensor("ebt", [G, 128, 2 * MT], F32, kind="ExternalInput")
    y_d = nc.dram_tensor("y", [T, DIM], F32, kind="ExternalOutput")

    x_flat = x_d.ap().rearrange("b n c -> (b n) c")

    # token tiles across all batches (for phases A and D)
    t_tiles = [(i * 128, min(128, T - i * 128)) for i in range((T + 127) // 128)]

    with tile.TileContext(nc) as tc:
        with (
            tc.tile_pool(name="consts", bufs=1) as consts,
            tc.tile_pool(name="vp", bufs=1) as vp,
            tc.tile_pool(name="dram", bufs=1, space="DRAM") as dram,
        ):
            aTdir_cm = tc.tile_pool(name="aTdir", bufs=1)
            aTdir_p = aTdir_cm.__enter__()
            xTp_cm = tc.tile_pool(name="xTp", bufs=1)
            xTp = xTp_cm.__enter__()
            aTdir = {}
            for g in (4, 5):
                aTdir[g] = aTdir_p.tile([128, T], F32R, tag=f"aTd_sb{g}",
                                        name=f"aTd_sb{g}")
            id32 = consts.tile([128, 128], F32R, tag="id32", name="id32")
            id16 = consts.tile([128, 128], BF16, tag="id16", name="id16")
            bb = consts.tile([128, DIM], F32, tag="bb", name="bb")
            ones_c = consts.tile([128, 1], F32, tag="ones_c", name="ones_c")
            zeros_c = consts.tile([128, 1], F32, tag="zeros_c", name="zeros_c")
            nc.vector.memset(ones_c[:], 1.0)
            nc.vector.memset(zeros_c[:], 0.0)
            nc.sync.dma_start(id32[:], id32_d.ap())
            nc.sync.dma_start(id16[:], id16_d.ap())
            nc.sync.dma_start(bb[:], bb_d.ap())

            xT = [xTp.tile([128, T], F32R, tag=f"xT{ci}", name=f"xT{ci}")
                  for ci in range(6)]
            qTd = [dram.tile([128, T], F32R, tag=f"qTd{g}", name=f"qTd{g}")
                   for g in range(G)]
            kTd = [dram.tile([128, B_PER * N_TOK + 128], F32R,
                             tag=f"kTd{g}", name=f"kTd{g}")
                   for g in range(G)]
            aTd = [dram.tile([128, T], F32R, tag=f"aTd{g}", name=f"aTd{g}")
                   for g in range(G)]

            # ---------------- Phase A: xT = transpose(x) ----------------
            pA_sb = tc.tile_pool(name="xload", bufs=3)
            pA_ps = tc.tile_pool(name="psA", bufs=6, space="PSUM")
            with pA_sb as xload, pA_ps as psA:
                for (t0, ts) in t_tiles:
                    xin = xload.tile([128, DIM], F32R, tag="xin", name="xin")
                    nc.scalar.dma_start(xin[:ts, :], x_flat[t0:t0 + ts, :])
                    for ci in range(6):
                        pt = psA.tile([128, 128], F32R, tag="pA", name="pA")
                        nc.tensor.transpose(
                            pt[:, :ts],
                            xin[:ts, ci * 128:(ci + 1) * 128],
                            id32[:ts, :ts],
                        )
                        if ci % 2 == 0:
                            nc.vector.tensor_copy(xT[ci][:, t0:t0 + ts], pt[:, :ts])
                        else:
                            nc.scalar.copy(xT[ci][:, t0:t0 + ts], pt[:, :ts])

            v_t = [[None] * MT for _ in range(B_PER)]
            # ---------------- Phase B2: qT / kT -> HBM scratch ----------------
            pB2_sb = tc.tile_pool(name="wqkp", bufs=1)
            pB2_bn = tc.tile_pool(name="b2bounce", bufs=1)
            pB2_ps = tc.tile_pool(name="psB2", bufs=6, space="PSUM")
            with pB2_sb as wqkp, pB2_bn as b2bn, pB2_ps as psB2:
                wqk = [wqkp.tile([128, 2 * DIM], F32R, tag=f"wqk{ci}", name=f"wqk{ci}")
                       for ci in range(6)]
                for ci in range(6):
                    nc.sync.dma_start(wqk[ci][:], wqk_d.ap()[ci * 128:(ci + 1) * 128, :])
                for g in range(G):
                    for part in ("q", "k"):
                        ot_col = g * 128 if part == "q" else DIM + g * 128
                        wid = B_PER * N_TOK + (128 if part == "k" else 0)
                        bn = b2bn.tile([128, wid], F32R,
                                       tag=f"bn{part}", name=f"bn{part}")
                        if part == "k":
                            nc.vector.tensor_copy(
                                bn[:, B_PER * N_TOK:],
                                zeros_c[:].broadcast_to((128, 128)))
                        for b in range(B_PER):
                            ps = psB2.tile([128, 512], F32, tag="psqk", name="psqk")
                            for ci in range(6):
                                nc.tensor.matmul(
                                    ps[:], wqk[ci][:, ot_col:ot_col + 128],
                                    xT[ci][:, b * N_TOK:b * N_TOK + 512],
                                    start=(ci == 0), stop=(ci == 5))
                            nc.vector.tensor_copy(
                                bn[:, b * N_TOK:b * N_TOK + 512], ps[:])
                        # all 4 batches' tail queries (n=512) in one free=4 pass
                        pst = psB2.tile([128, 512], F32, tag="psqk", name="psqkt")
                        for ci in range(6):
                            nc.tensor.matmul(
                                pst[:, 0:B_PER],
                                wqk[ci][:, ot_col:ot_col + 128].bitcast(F32),
                                xT[ci].rearrange("p (b n) -> p b n", n=N_TOK)[:, :, 512].bitcast(F32),
                                start=(ci == 0), stop=(ci == 5))
                        nc.vector.tensor_copy(
                            bn.rearrange("p (b n) -> p b n", n=N_TOK)[:, :, 512]
                            if part == "q" else
                            bn[:, 0:B_PER * N_TOK].rearrange("p (b n) -> p b n", n=N_TOK)[:, :, 512],
                            pst[:, 0:B_PER])
                        dst = qTd[g] if part == "q" else kTd[g]
                        nc.sync.dma_start(dst[:], bn[:])

            # ---------------- Phase B: v (token-major, 65-col head groups) ----
            pB_sb = tc.tile_pool(name="wvp", bufs=1)
            pB_ps = tc.tile_pool(name="psB", bufs=3, space="PSUM")
            with pB_sb as wvp, pB_ps as psB:
                wv = [wvp.tile([128, DIM], F32R, tag=f"wv{ci}", name=f"wv{ci}")
                      for ci in range(6)]
                for ci in range(6):
                    nc.scalar.dma_start(wv[ci][:], wv_d.ap()[ci * 128:(ci + 1) * 128, :])
                for b in range(B_PER):
                    for mt in range(MT):
                        vt = vp.tile([128, HEADS * (HD + 1)], F32R,
                                     tag=f"v{b}_{mt}", name=f"v{b}_{mt}")
                        v3 = vt.rearrange("p (h x) -> p h x", x=HD + 1)
                        if mt == MT - 1:
                            nc.vector.tensor_copy(
                                vt[:], zeros_c[:].broadcast_to((128, HEADS * (HD + 1))))
                        nc.vector.tensor_copy(
                            v3[:, :, HD:HD + 1],
                            ones_c[:].broadcast_to((128, HEADS, 1)))
                        v_t[b][mt] = vt
                        if mt == MT - 1:
                            continue
                        psv = psB.tile([128, 1024], F32, tag="psv", name="psv")
                        lhs0 = b * N_TOK + mt * 128
                        for ci in range(6):
                            lhsT = xT[ci][:, lhs0:lhs0 + 128]
                            nc.tensor.matmul(psv[:, 0:512], lhsT, wv[ci][:, 0:512],
                                             start=(ci == 0), stop=(ci == 5))
                            nc.tensor.matmul(psv[:, 512:768], lhsT, wv[ci][:, 512:768],
                                             start=(ci == 0), stop=(ci == 5))
                        src = psv[:, 0:DIM].rearrange("p (h d) -> p h d", d=HD)
                        nc.vector.tensor_copy(v3[:, :, 0:HD], src)
                # all 4 batches' tail tokens (n=512) in one M=4 pass
                psvt = psB.tile([128, 1024], F32, tag="psvt", name="psvt", bufs=1)
                for ci in range(6):
                    lhsT = xT[ci].rearrange("p (b n) -> p b n", n=N_TOK)[:, :, 512]
                    nc.tensor.matmul(psvt[:B_PER, 0:512], lhsT, wv[ci][:, 0:512],
                                     start=(ci == 0), stop=(ci == 5))
                    nc.tensor.matmul(psvt[:B_PER, 512:768], lhsT, wv[ci][:, 512:768],
                                     start=(ci == 0), stop=(ci == 5))
                vstage = vp.tile([B_PER, HEADS * (HD + 1)], F32R,
                                 tag="vstage", name="vstage")
                vs3 = vstage.rearrange("p (h x) -> p h x", x=HD + 1)
                nc.vector.tensor_copy(
                    vs3[:, :, 0:HD],
                    psvt[0:B_PER, 0:DIM].rearrange("p (h d) -> p h d", d=HD))
                for b in range(B_PER):
                    v3t = v_t[b][MT - 1].rearrange("p (h x) -> p h x", x=HD + 1)
                    nc.sync.dma_start(v3t[0:1, :, 0:HD], vs3[b:b + 1, :, 0:HD])

            xTp_cm.__exit__(None, None, None)
            _phases = os.environ.get("K_PHASES", "full")
            _cvar = os.environ.get("K_CVAR", "full")
            _h2s = (0,) if _cvar == "nohi" else (0, 1)
            _tails = _cvar not in ("notail", "min")
            _donorm = _cvar not in ("nonorm", "min")
            _hibase = 0 if _cvar == "hi0" else 64
            # ---------------- Phase C: attention ----------------
            if _phases != "ab2":
                pC1 = tc.tile_pool(name="qgp", bufs=2)
                pC2 = tc.tile_pool(name="kgp", bufs=2)
                pC3 = tc.tile_pool(name="btp", bufs=2)
                pC4 = tc.tile_pool(name="pmp", bufs=3)
                pC5 = tc.tile_pool(name="smallp", bufs=2)
                pC6 = tc.tile_pool(name="atbp", bufs=3)
                pS_ps = tc.tile_pool(name="psS", bufs=2, space="PSUM")
                pSt_ps = tc.tile_pool(name="psSt", bufs=1, space="PSUM")
                pU_ps = tc.tile_pool(name="psU", bufs=1, space="PSUM")
                pUt_ps = tc.tile_pool(name="psUt", bufs=1, space="PSUM")
                pC7 = tc.tile_pool(name="usbp", bufs=2)
                with pC1 as qgp, pC2 as kgp, pC3 as btp, pC4 as pmp, \
                     pC5 as smallp, pC6 as atbp, pC7 as usbp, \
                     pS_ps as psS, pSt_ps as psSt, pU_ps as psU, pUt_ps as psUt:
                    for g in range(G):
                        qgs, kgs = [], []
                        for h2 in range(2):
                            qgt = qgp.tile([64, T], F32R, tag=f"qg{h2}", name=f"qg{h2}")
                            nc.gpsimd.dma_start(qgt[:], qTd[g][h2 * 64:h2 * 64 + 64, :])
                            qgs.append(qgt)
                            kgt = kgp.tile([64, B_PER * N_TOK + 128], F32R,
                                           tag=f"kg{h2}", name=f"kg{h2}")
                            nc.gpsimd.dma_start(kgt[:], kTd[g][h2 * 64:h2 * 64 + 64, :])
                            kgs.append(kgt)
                        btpair = btp.tile([128, 2, MT, N_TOK], BF16,
                                          tag="btpair", name="btpair")
                        nc.gpsimd.dma_start(
                            btpair[:],
                            bT_d.ap()[2 * g:2 * g + 2].rearrange("h p m n -> p h m n"))
                        bt = [btpair[:, 0], btpair[:, 1]]
                        ebg = btp.tile([128, 2 * MT], F32, tag="ebg", name="ebg")
                        nc.gpsimd.dma_start(ebg[:], ebt_d.ap()[g])
                        for b in range(B_PER):
                            U = psU.tile([HD + 1, 1024], F32, tag="U", name="U")
                            Ut = psUt.tile([HD + 1, 2 * MT], F32, tag="Ut", name="Ut")
                            stail = psSt.tile([128, 2 * MT], F32, tag="st", name="st")
                            praw = smallp.tile([128, 2 * MT], F32, tag="praw", name="praw")
                            ptail = smallp.tile([128, 2 * MT], F32R, tag="pt", name="pt")
                            # tail-query (n = 512) logits: single-shot groups per col
                            for mt in (range(MT) if _tails else ()):
                                for h2 in _h2s:
                                    tci = mt * 2 + h2
                                    r0 = h2 * _hibase
                                    nc.tensor.matmul(
                                        stail[:, tci:tci + 1],
                                        kgs[h2][:,
                                           b * N_TOK + mt * 128:b * N_TOK + mt * 128 + 128].bitcast(F32),
                                        qgs[h2][:,
                                           b * N_TOK + 512:b * N_TOK + 513].bitcast(F32),
                                        start=True, stop=True)
                            if _tails:
                                nc.scalar.activation(praw[:], stail[:], AF.Exp)
                                nc.vector.tensor_mul(ptail[:], praw[:], ebg[:])
                            def emit_av(mt, pm):
                                st = (mt == 0)
                                sp = (mt == MT - 1)
                                va0 = v_t[b][mt][:, (2 * g) * (HD + 1):(2 * g) * (HD + 1) + HD + 1]
                                va1 = v_t[b][mt][:, (2 * g + 1) * (HD + 1):(2 * g + 1) * (HD + 1) + HD + 1]
                                nc.tensor.matmul(U[:, 0:512], va0, pm[:, 0:512],
                                                 start=st, stop=sp)
                                if 1 in _h2s:
                                    nc.tensor.matmul(U[:, 512:1024], va1, pm[:, 512:1024],
                                                     start=st, stop=sp)
                                if _tails:
                                    nc.tensor.matmul(Ut[:, 2 * mt:2 * mt + 1],
                                                     va0.bitcast(F32),
                                                     ptail[:, 2 * mt:2 * mt + 1].bitcast(F32),
                                                     start=True, stop=True)
                                    if 1 in _h2s:
                                        nc.tensor.matmul(Ut[:, 2 * mt + 1:2 * mt + 2],
                                                         va1.bitcast(F32),
                                                         ptail[:, 2 * mt + 1:2 * mt + 2].bitcast(F32),
                                                         start=True, stop=True)

                            prev_av = None
                            for mt in range(MT):
                                sm = psS.tile([128, 1024], F32, tag="S", name="S")
                                pm = pmp.tile([128, 1024], F32R, tag="pm", name="pm")
                                for h2 in _h2s:
                                    c0 = h2 * 512
                                    r0 = h2 * _hibase
                                    nc.tensor.matmul(
                                        sm[:, c0:c0 + 512], id16[:],
                                        bt[h2][:, mt, 0:512],
                                        start=True, stop=False)
                                    nc.tensor.matmul(
                                        sm[:, c0:c0 + 512],
                                        kgs[h2][:,
                                           b * N_TOK + mt * 128:b * N_TOK + mt * 128 + 128],
                                        qgs[h2][:, b * N_TOK:b * N_TOK + 512],
                                        start=False, stop=True)
                                nc.scalar.activation(pm[:], sm[:], AF.Exp)
                                if prev_av is not None:
                                    emit_av(*prev_av)
                                prev_av = (mt, pm)
                            emit_av(*prev_av)
                            if g >= 4:
                                aTb = aTdir[g][:, b * N_TOK:(b + 1) * N_TOK]
                            else:
                                aTb = atbp.tile([128, N_TOK], F32R, tag="atb", name="atb")
                            Usb = usbp.tile([HD + 1, 1024], F32, tag="usb", name="Usb")
                            nc.vector.tensor_copy(Usb[:], U[:, 0:1024])
                            Ut3 = Ut.rearrange("p (m h) -> p m h", h=2)
                            for h2 in _h2s:
                                r0 = h2 * 512
                                if not _donorm:
                                    nc.vector.tensor_copy(
                                        aTb[h2 * 64:h2 * 64 + 64, 0:512],
                                        U[0:HD, r0:r0 + 512])
                                    continue
                                rn = smallp.tile([1, 512], F32, tag="rn", name="rn")
                                nc.vector.reciprocal(rn[:], Usb[HD:HD + 1, r0:r0 + 512])
                                bc = smallp.tile([HD, 512], F32, tag="bc", name="bc")
                                nc.gpsimd.partition_broadcast(bc[:], rn[:])
                                nc.vector.tensor_mul(
                                    aTb[h2 * 64:h2 * 64 + 64, 0:512],
                                    Usb[0:HD, r0:r0 + 512], bc[:])
                                if not _tails:
                                    continue
                                # tail query column
                                utr = smallp.tile([HD + 1, 1], F32, tag="utr", name="utr")
                                nc.vector.tensor_reduce(
                                    utr[:], Ut3[:, :, h2:h2 + 1],
                                    axis=mybir.AxisListType.XY, op=mybir.AluOpType.add)
                                rnt = smallp.tile([1, 1], F32, tag="rnt", name="rnt")
                                nc.vector.reciprocal(rnt[:], utr[HD:HD + 1, :])
                                bct = smallp.tile([HD, 1], F32, tag="bct", name="bct")
                                nc.gpsimd.partition_broadcast(bct[:], rnt[:])
                                nc.vector.tensor_mul(
                                    aTb[h2 * 64:h2 * 64 + 64, 512:513],
                                    utr[0:HD, :], bct[:])
                            if g < 4:
                                nc.sync.dma_start(
                                    aTd[g][:, b * N_TOK:(b + 1) * N_TOK], aTb[:])

            # ---------------- Phase D: y = a @ Wp^T + b ----------------
            if _phases == "full":
                pD1 = tc.tile_pool(name="aTp", bufs=1)
                pD2 = tc.tile_pool(name="wpp", bufs=1)
                pD3 = tc.tile_pool(name="ytp", bufs=3)
                pD_ps = tc.tile_pool(name="psD", bufs=4, space="PSUM")
                with pD1 as aTp, pD2 as wpp, pD3 as ytp, pD_ps as psD:
                    aT = [aTp.tile([128, T], F32R, tag=f"aT{ci}", name=f"aT{ci}")
                          if ci < 4 else aTdir[ci] for ci in range(6)]
                    wpal = wpp.tile([128, 6, DIM], F32R, tag="wpal", name="wpal")
                    wp_src = wp_d.ap().rearrange("(c p) d -> p c d", p=128)
                    nc.scalar.dma_start(wpal[:, 4:6], wp_src[:, 4:6])
                    nc.scalar.dma_start(wpal[:, 0:4], wp_src[:, 0:4])
                    wp = [wpal[:, ci] for ci in range(6)]
                    for ci in range(4):
                        nc.sync.dma_start(aT[ci][:], aTd[ci][:])
                    ci_order = (4, 5, 0, 1, 2, 3)
                    for (t0, ts) in t_tiles:
                        psy = psD.tile([128, 1024], F32, tag="psy", name="psy")
                        for j, ci in enumerate(ci_order):
                            lhsT = aT[ci][:, t0:t0 + ts]
                            nc.tensor.matmul(psy[:ts, 0:512], lhsT, wp[ci][:, 0:512],
                                             start=(j == 0), stop=(j == 5))
                            nc.tensor.matmul(psy[:ts, 512:768], lhsT, wp[ci][:, 512:768],
                                             start=(j == 0), stop=(j == 5))
                        yt = ytp.tile([128, DIM], F32, tag="yt", name="yt")
                        nc.vector.tensor_add(yt[:ts, :], psy[:ts, 0:DIM], bb[:ts, :])
                        nc.sync.dma_start(y_d.ap()[t0:t0 + ts, :], yt[:ts, :])
            aTdir_cm.__exit__(None, None, None)

    nc.compile()
    return nc


def get_nc():
    if "nc" not in _CACHE:
        _CACHE["nc"] = _build_nc()
    return _CACHE["nc"]


def host_prep(w_qkv, bias_table, w_proj, b_proj, rel_index):
    """Host-side packing shared by all cores."""
    import ml_dtypes
    w_qkv = np.asarray(w_qkv, dtype=np.float32)
    wq = w_qkv[0:DIM] * np.float32(SCALE)
    wk = w_qkv[DIM:2 * DIM]
    wqkT = np.ascontiguousarray(np.concatenate([wq, wk], axis=0).T)
    wvT = np.ascontiguousarray(w_qkv[2 * DIM:3 * DIM].T)
    wpT = np.ascontiguousarray(np.asarray(w_proj, dtype=np.float32).T)
    tbl = np.asarray(bias_table, dtype=np.float32)
    gat = tbl[np.asarray(rel_index)]            # [n, m, h]
    BmT = gat.transpose(2, 1, 0)                # [h, m, n] = B^T
    Bm = np.full((HEADS, MT * 128, N_TOK), NEG, dtype=np.float32)
    Bm[:, :N_TOK, :] = BmT
    bT = np.ascontiguousarray(
        Bm.reshape(HEADS, MT, 128, N_TOK).transpose(0, 2, 1, 3)
    ).astype(ml_dtypes.bfloat16)
    id32 = np.eye(128, dtype=np.float32)
    id16 = np.eye(128, dtype=np.float32).astype(ml_dtypes.bfloat16)
    bb = np.ascontiguousarray(
        np.broadcast_to(np.asarray(b_proj, dtype=np.float32), (128, DIM)))
    # tail-query (n=512) exp(bias) factors: ebt[g, p, mt*2+h2]
    ebt = np.zeros((G, 128, 2 * MT), dtype=np.float32)
    with np.errstate(under="ignore"):
        for g in range(G):
            for mt in range(MT):
                for h2 in range(2):
                    ebt[g, :, 2 * mt + h2] = np.exp(
                        Bm[2 * g + h2, mt * 128:(mt + 1) * 128, 512])
    return {"wqkT": wqkT, "wvT": wvT, "wpT": wpT, "bT": bT,
            "id32": id32, "id16": id16, "bb": bb, "ebt": ebt}


def kernel(x, w_qkv, bias_table, w_proj, b_proj, rel_index):
    import time
    from concourse.bass_utils import run_bass_kernel_spmd

    x = np.asarray(x, dtype=np.float32)
    shared = host_prep(w_qkv, bias_table, w_proj, b_proj, rel_index)
    nc = get_nc()
    in_maps = []
    for c in range(N_CORES):
        m = {"x": np.ascontiguousarray(x[c * B_PER:(c + 1) * B_PER])}
        m.update(shared)
        in_maps.append(m)
    # Transient NRT_EXEC_UNIT_UNRECOVERABLE failures have been observed on
    # this fabric; an identical retry passes, so guard the execution.
    last_exc = None
    for attempt in range(3):
        try:
            res = run_bass_kernel_spmd(nc, in_maps, core_ids=list(range(N_CORES)))
            break
        except Exception as e:
            last_exc = e
            time.sleep(2.0)
    else:
        raise last_exc
    out = np.concatenate(
        [res.results[c]["y"].reshape(B_PER, N_TOK, DIM) for c in range(N_CORES)],
        axis=0,
    )
    return out

